# revision 8
# baseline (speedup 1.0000x reference)
"""GQA attention kernel for Trainium2, 8 NeuronCores, head-per-core sharding.

Per core c (head h=c, kv-group g=c//4):
  phase A: k/v projections for all L rows (k: rmsnorm+rope, transposed to kT)
  phase B: per 512-query tile: q/gate projection (rmsnorm+rope+sigmoid,
           transposed), then S^T = kT.T@qT flash attention with exp on ACT,
           causal fill via gpsimd affine_select, denominator via ones-matmul,
           AV into OT, gating, out-projection with 1/denom folded into the
           PSUM->SBUF copy scale. Host sums the 8 partial outputs.
All matmuls run in float32r (1 cycle/row at moving-dim>=256).
"""
import sys
sys.path.insert(0, '/opt/trn_rl_repo')
import numpy as np
import concourse.bacc as bacc
import concourse.mybir as mybir
from concourse.tile import TileContext
from concourse.bass_utils import run_bass_kernel_spmd

P = 128
IDIM = 2048
D = 256           # head dim
ROPE = 64
TQ = 512          # queries per attention tile
EPS = 1e-6
F32 = mybir.dt.float32
F32R = mybir.dt.float32r
AF = mybir.ActivationFunctionType
AX = mybir.AxisListType
OP = mybir.AluOpType

_nc_cache = {}
_last_in_maps = None


def build_nc(L, causal):
    nc = bacc.Bacc()
    nKO = IDIM // P   # 16 idim chunks
    nL = L // P       # 128-row L tiles
    nQT = L // TQ     # query tiles
    nLS = TQ // P     # 4 L-subtiles per query tile

    xT = nc.declare_dram_parameter("xT", [IDIM, L], F32R, isOutput=False)
    wqg = nc.declare_dram_parameter("wqg", [IDIM, 2 * D], F32R, isOutput=False)
    wk = nc.declare_dram_parameter("wk", [IDIM, D], F32R, isOutput=False)
    wv = nc.declare_dram_parameter("wv", [IDIM, D], F32R, isOutput=False)
    wo = nc.declare_dram_parameter("wo", [D, IDIM], F32R, isOutput=False)
    cosq = nc.declare_dram_parameter("cosq", [L, ROPE], F32, isOutput=False)
    sinq = nc.declare_dram_parameter("sinq", [L, ROPE], F32, isOutput=False)
    cosk = nc.declare_dram_parameter("cosk", [L, ROPE], F32, isOutput=False)
    sink = nc.declare_dram_parameter("sink", [L, ROPE], F32, isOutput=False)
    qgam = nc.declare_dram_parameter("qgam", [P, 2], F32, isOutput=False)
    kgam = nc.declare_dram_parameter("kgam", [P, 2], F32, isOutput=False)
    ones = nc.declare_dram_parameter("ones", [P, 1], F32R, isOutput=False)
    ident = nc.declare_dram_parameter("ident", [P, P], F32, isOutput=False)
    y = nc.declare_dram_parameter("y", [L, IDIM], F32, isOutput=True)

    with TileContext(nc) as tc:
        with tc.tile_pool(name="persist", bufs=1) as persist, \
             tc.tile_pool(name="xts", bufs=3) as xts, \
             tc.tile_pool(name="scr", bufs=3) as scr, \
             tc.tile_pool(name="stats", bufs=4) as stats, \
             tc.tile_pool(name="rope", bufs=2) as ropep:

            # persistent tensors
            kT_all = persist.tile([P, 2, L], F32R)     # 32KB/part
            v_all = persist.tile([P, nL, D], F32R)     # 32KB/part
            wqg_sb = persist.tile([P, nKO, 2 * D], F32R)
            wo_sb = persist.tile([P, 2, IDIM], F32R)
            ident_sb = persist.tile([P, P], F32)
            ones_sb = persist.tile([P, 1], F32R)
            qgam_sb = persist.tile([P, 2], F32)
            kgam_sb = persist.tile([P, 2], F32)
            eps_k = persist.tile([P, 1], F32)
            eps_q = persist.tile([P, 1], F32)
            nc.gpsimd.memset(eps_k[:], EPS)
            nc.gpsimd.memset(eps_q[:], D * EPS)
            nc.sync.dma_start(wqg_sb[:], wqg.rearrange("(ko p) n -> p ko n", p=P))
            nc.sync.dma_start(wo_sb[:], wo.rearrange("(dc p) n -> p dc n", p=P))
            nc.sync.dma_start(ident_sb[:], ident[:, :])
            nc.sync.dma_start(ones_sb[:], ones[:, :])
            nc.sync.dma_start(qgam_sb[:], qgam[:, :])
            nc.sync.dma_start(kgam_sb[:], kgam[:, :])

            def norm_rope(psum_in, cos_t, sin_t, sqrt_scale, sqrt_bias):
                """rmsnorm + partial rope on a [P, D] natural-layout psum tile.
                Returns normed+roped [P, D] f32 sbuf tile."""
                sq = scr.tile([P, D], F32, tag="sq")
                nc.scalar.activation(sq[:], psum_in, AF.Square)
                ssq = stats.tile([P, 1], F32, tag="ssq")
                nc.vector.reduce_sum(ssq[:], sq[:], axis=AX.X)
                std = stats.tile([P, 1], F32, tag="std")
                nc.scalar.activation(std[:], ssq[:], AF.Sqrt,
                                     scale=sqrt_scale, bias=sqrt_bias)
                rstd = stats.tile([P, 1], F32, tag="rstd")
                nc.vector.reciprocal(rstd[:], std[:])
                xn = scr.tile([P, D], F32, tag="xn")
                nc.vector.tensor_scalar_mul(xn[:], psum_in, rstd[:])
                # rope on first 64 cols: xn' = xn*cos + rot(xn)*sin
                t1 = ropep.tile([P, ROPE], F32, tag="t1")
                nc.vector.tensor_mul(t1[:], xn[:, 0:ROPE], cos_t)
                rot = ropep.tile([P, ROPE], F32, tag="rot")
                nc.vector.tensor_scalar_mul(rot[:, 0:32], xn[:, 32:64], -1.0)
                nc.vector.tensor_copy(rot[:, 32:64], xn[:, 0:32])
                t2 = ropep.tile([P, ROPE], F32, tag="t2")
                nc.vector.tensor_mul(t2[:], rot[:], sin_t)
                nc.vector.tensor_add(xn[:, 0:ROPE], t1[:], t2[:])
                return xn

            # ---------------- phase A: k/v for all L ----------------
            with tc.tile_pool(name="wkv", bufs=1) as wkvp, \
                 tc.tile_pool(name="pk", bufs=2, space="PSUM") as pkp, \
                 tc.tile_pool(name="pv", bufs=2, space="PSUM") as pvp, \
                 tc.tile_pool(name="ptrA", bufs=2, space="PSUM") as ptrA:
                wk_sb = wkvp.tile([P, nKO, D], F32R)
                wv_sb = wkvp.tile([P, nKO, D], F32R)
                nc.sync.dma_start(wk_sb[:], wk.rearrange("(ko p) n -> p ko n", p=P))
                nc.sync.dma_start(wv_sb[:], wv.rearrange("(ko p) n -> p ko n", p=P))

                for t in range(nL):
                    xt = xts.tile([P, nKO, P], F32R, tag="xt")
                    nc.sync.dma_start(
                        xt[:], xT[:, t * P:(t + 1) * P].rearrange(
                            "(ko p) l -> p ko l", p=P))
                    psum_k = pkp.tile([P, D], F32)
                    psum_v = pvp.tile([P, D], F32)
                    for ko in range(nKO):
                        nc.tensor.matmul(psum_k[:], xt[:, ko], wk_sb[:, ko],
                                         start=(ko == 0), stop=(ko == nKO - 1))
                    for ko in range(nKO):
                        nc.tensor.matmul(psum_v[:], xt[:, ko], wv_sb[:, ko],
                                         start=(ko == 0), stop=(ko == nKO - 1))
                    nc.scalar.copy(v_all[:, t], psum_v[:])
                    ck = ropep.tile([P, ROPE], F32, tag="ck")
                    sk = ropep.tile([P, ROPE], F32, tag="sk")
                    nc.sync.dma_start(ck[:], cosk[t * P:(t + 1) * P, :])
                    nc.sync.dma_start(sk[:], sink[t * P:(t + 1) * P, :])
                    kn = norm_rope(psum_k[:], ck[:], sk[:], 1.0 / D, eps_k[:])
                    for dc in range(2):
                        ptr = ptrA.tile([P, P], F32, tag="ptr")
                        nc.tensor.transpose(ptr[:], kn[:, dc * P:(dc + 1) * P],
                                            ident_sb[:])
                        nc.scalar.activation(kT_all[:, dc, t * P:(t + 1) * P],
                                             ptr[:], AF.Copy,
                                             scale=kgam_sb[:, dc:dc + 1])

            # ---------------- phase B: per query tile ----------------
            with tc.tile_pool(name="qgp", bufs=2, space="PSUM") as qgyp, \
                 tc.tile_pool(name="ptrB", bufs=1, space="PSUM") as ptrB, \
                 tc.tile_pool(name="ps", bufs=2, space="PSUM") as psp, \
                 tc.tile_pool(name="pot", bufs=1, space="PSUM") as potp, \
                 tc.tile_pool(name="pden", bufs=1, space="PSUM") as pdenp, \
                 tc.tile_pool(name="qt", bufs=2) as qtp, \
                 tc.tile_pool(name="es", bufs=3) as esp, \
                 tc.tile_pool(name="og", bufs=2) as ogp, \
                 tc.tile_pool(name="yo", bufs=3) as yop:
                for qt in range(nQT):
                    qT_t = qtp.tile([P, 2, TQ], F32R, tag="qT")
                    sgT_t = qtp.tile([P, 2, TQ], F32, tag="sgT")
                    for ls in range(nLS):
                        t = qt * nLS + ls
                        xt = xts.tile([P, nKO, P], F32R, tag="xt")
                        nc.sync.dma_start(
                            xt[:], xT[:, t * P:(t + 1) * P].rearrange(
                                "(ko p) l -> p ko l", p=P))
                        psum_qg = qgyp.tile([P, 2 * D], F32, tag="qg")
                        for ko in range(nKO):
                            nc.tensor.matmul(psum_qg[:], xt[:, ko], wqg_sb[:, ko],
                                             start=(ko == 0), stop=(ko == nKO - 1))
                        # gate -> sigmoid -> transpose
                        sg = scr.tile([P, D], F32, tag="sg")
                        nc.scalar.activation(sg[:], psum_qg[:, D:2 * D], AF.Sigmoid)
                        for dc in range(2):
                            ptr = ptrB.tile([P, P], F32, tag="ptr")
                            nc.tensor.transpose(ptr[:], sg[:, dc * P:(dc + 1) * P],
                                                ident_sb[:])
                            nc.scalar.copy(sgT_t[:, dc, ls * P:(ls + 1) * P], ptr[:])
                        # q: rmsnorm (with /16 folded) + rope + transpose
                        cq = ropep.tile([P, ROPE], F32, tag="cq")
                        sq_ = ropep.tile([P, ROPE], F32, tag="sq_")
                        nc.sync.dma_start(cq[:], cosq[t * P:(t + 1) * P, :])
                        nc.sync.dma_start(sq_[:], sinq[t * P:(t + 1) * P, :])
                        # sqrt(ssq + D*eps) = sqrt(D) * sqrt(mean+eps); recip
                        # gives rstd/16 since sqrt(D)=16
                        qn = norm_rope(psum_qg[:, 0:D], cq[:], sq_[:], 1.0, eps_q[:])
                        for dc in range(2):
                            ptr = ptrB.tile([P, P], F32, tag="ptr")
                            nc.tensor.transpose(ptr[:], qn[:, dc * P:(dc + 1) * P],
                                                ident_sb[:])
                            nc.scalar.activation(qT_t[:, dc, ls * P:(ls + 1) * P],
                                                 ptr[:], AF.Copy,
                                                 scale=qgam_sb[:, dc:dc + 1])

                    # attention over kv chunks
                    n_kc = (qt + 1) * nLS if causal else nL
                    psum_ot = [potp.tile([P, TQ], F32, tag=f"ot{dc}", name=f"ot{dc}")
                               for dc in range(2)]
                    psum_den = pdenp.tile([1, TQ], F32, tag="den")
                    for kc in range(n_kc):
                        psum_s = psp.tile([P, TQ], F32, tag="s")
                        for dc in range(2):
                            nc.tensor.matmul(
                                psum_s[:], kT_all[:, dc, kc * P:(kc + 1) * P],
                                qT_t[:, dc], start=(dc == 0), stop=(dc == 1))
                        es = esp.tile([P, TQ], F32R, tag="es")
                        nc.scalar.activation(es[:], psum_s[:], AF.Exp)
                        if causal and kc >= qt * nLS:
                            j = kc - qt * nLS
                            # keep where qf - p - 128*j >= 0 else 0
                            nc.gpsimd.affine_select(
                                out=es[:], in_=es[:], compare_op=OP.is_ge,
                                fill=0.0, base=-P * j,
                                pattern=[[1, TQ]], channel_multiplier=-1)
                        nc.tensor.matmul(psum_den[:], ones_sb[:],
                                         es[:].rearrange("k (ls p) -> k p ls", p=P),
                                         start=(kc == 0), stop=(kc == n_kc - 1))
                        for dc in range(2):
                            nc.tensor.matmul(
                                psum_ot[dc][:], v_all[:, kc, dc * P:(dc + 1) * P],
                                es[:], start=(kc == 0), stop=(kc == n_kc - 1))

                    # denominators -> [P, nLS] reciprocal via dram bounce
                    den_sb = stats.tile([1, TQ], F32, tag="densb")
                    nc.vector.tensor_copy(den_sb[:], psum_den[:])
                    den_in = stats.tile([P, nLS], F32, tag="denin")
                    nc.sync.dma_start(den_in[:, :], den_sb[0:1, :])
                    recip = stats.tile([P, nLS], F32, tag="recip")
                    nc.vector.reciprocal(recip[:], den_in[:])

                    # gate the attention output
                    og = [ogp.tile([P, TQ], F32R, tag=f"og{dc}", name=f"og{dc}")
                          for dc in range(2)]
                    for dc in range(2):
                        nc.vector.tensor_mul(og[dc][:], psum_ot[dc][:],
                                             sgT_t[:, dc])

                    # out-projection, 1/den folded into copy scale
                    for ls in range(nLS):
                        for oc in range(IDIM // TQ):
                            psum_y = psp.tile([P, TQ], F32, tag="s", name="psum_y")
                            for dc in range(2):
                                nc.tensor.matmul(
                                    psum_y[:], og[dc][:, ls * P:(ls + 1) * P],
                                    wo_sb[:, dc, oc * TQ:(oc + 1) * TQ],
                                    start=(dc == 0), stop=(dc == 1))
                            y_sb = yop.tile([P, TQ], F32, tag="ysb")
                            nc.scalar.activation(y_sb[:], psum_y[:], AF.Copy,
                                                 scale=recip[:, ls:ls + 1])
                            nc.sync.dma_start(
                                y[qt * TQ + ls * P: qt * TQ + (ls + 1) * P,
                                  oc * TQ:(oc + 1) * TQ], y_sb[:])
    nc.compile()
    return nc


def _get_nc(L, causal):
    key = (L, causal)
    if key not in _nc_cache:
        _nc_cache[key] = build_nc(L, causal)
    return _nc_cache[key]


def kernel(x, cos, sin, mask, wq, wk, wv, wo, q_gamma, k_gamma):
    B, L, _ = x.shape
    n_heads = 8
    group_size = 4

    mask = np.asarray(mask)
    causal_ref = np.triu(np.ones((L, L), dtype=bool), k=1)
    if np.array_equal(mask, causal_ref):
        causal = True
    elif not mask.any():
        causal = False
    else:
        raise NotImplementedError("only causal or empty masks supported")

    nc = _get_nc(L, causal)

    xT = np.ascontiguousarray(np.asarray(x[0]).T, dtype=np.float32)
    cos = np.asarray(cos, dtype=np.float32)
    sin = np.asarray(sin, dtype=np.float32)
    qg = np.asarray(q_gamma, dtype=np.float32)
    kg = np.asarray(k_gamma, dtype=np.float32)
    idx = (np.arange(ROPE) + 32) % ROPE

    def fold(g):
        cosA = np.ascontiguousarray(cos * g[None, :ROPE])
        sinA = np.ascontiguousarray(sin * g[idx][None, :])
        gam = np.empty((P, 2), dtype=np.float32)
        gam[:, 0] = np.concatenate([np.ones(ROPE, np.float32), g[ROPE:P]])
        gam[:, 1] = g[P:D]
        return cosA, sinA, gam

    cosq_h, sinq_h, qgam_h = fold(qg)
    cosk_h, sink_h, kgam_h = fold(kg)
    ones_h = np.ones((P, 1), dtype=np.float32)
    ident_h = np.eye(P, dtype=np.float32)

    in_maps = []
    for c in range(n_heads):
        g = c // group_size
        in_maps.append({
            "xT": xT,
            "wqg": np.ascontiguousarray(wq[:, c * 2 * D:(c + 1) * 2 * D]),
            "wk": np.ascontiguousarray(wk[:, g * D:(g + 1) * D]),
            "wv": np.ascontiguousarray(wv[:, g * D:(g + 1) * D]),
            "wo": np.ascontiguousarray(wo[c * D:(c + 1) * D, :]),
            "cosq": cosq_h, "sinq": sinq_h,
            "cosk": cosk_h, "sink": sink_h,
            "qgam": qgam_h, "kgam": kgam_h,
            "ones": ones_h, "ident": ident_h,
        })
    global _last_in_maps
    _last_in_maps = in_maps
    res = run_bass_kernel_spmd(nc, in_maps, core_ids=list(range(n_heads)))
    out = np.zeros((L, IDIM), dtype=np.float64)
    for c in range(n_heads):
        out += res.results[c]["y"]
    return out.astype(np.float32).reshape(B, L, IDIM)


# revision 9
# speedup vs baseline: 1.0454x; 1.0454x over previous
"""GQA attention kernel for Trainium2, 8 NeuronCores, head-per-core sharding.

Per core c (head h=c, kv-group g=c//4):
  phase A: k/v projections for all L rows (k: rmsnorm+rope, transposed to kT)
  phase B: per 512-query tile: q/gate projection (rmsnorm+rope+sigmoid,
           transposed), then S^T = kT.T@qT flash attention with exp on ACT,
           causal fill via gpsimd affine_select, denominator via ones-matmul,
           AV into OT, gating, out-projection with 1/denom folded into the
           PSUM->SBUF copy scale. Host sums the 8 partial outputs.
All matmuls run in float32r (1 cycle/row at moving-dim>=256).
"""
import sys
sys.path.insert(0, '/opt/trn_rl_repo')
import numpy as np
import concourse.bacc as bacc
import concourse.mybir as mybir
from concourse.tile import TileContext
from concourse.bass_utils import run_bass_kernel_spmd

P = 128
IDIM = 2048
D = 256           # head dim
ROPE = 64
TQ = 512          # queries per attention tile
EPS = 1e-6
F32 = mybir.dt.float32
F32R = mybir.dt.float32r
AF = mybir.ActivationFunctionType
AX = mybir.AxisListType
OP = mybir.AluOpType

_nc_cache = {}
_last_in_maps = None


def build_nc(L, causal):
    nc = bacc.Bacc()
    nKO = IDIM // P   # 16 idim chunks
    nL = L // P       # 128-row L tiles
    nQT = L // TQ     # query tiles
    nLS = TQ // P     # 4 L-subtiles per query tile

    xT = nc.declare_dram_parameter("xT", [IDIM, L], F32R, isOutput=False)
    wqg = nc.declare_dram_parameter("wqg", [IDIM, 2 * D], F32R, isOutput=False)
    wk = nc.declare_dram_parameter("wk", [IDIM, D], F32R, isOutput=False)
    wv = nc.declare_dram_parameter("wv", [IDIM, D], F32R, isOutput=False)
    wo = nc.declare_dram_parameter("wo", [D, IDIM], F32R, isOutput=False)
    cosq = nc.declare_dram_parameter("cosq", [L, ROPE], F32, isOutput=False)
    sinq = nc.declare_dram_parameter("sinq", [L, ROPE], F32, isOutput=False)
    cosk = nc.declare_dram_parameter("cosk", [L, ROPE], F32, isOutput=False)
    sink = nc.declare_dram_parameter("sink", [L, ROPE], F32, isOutput=False)
    qgam = nc.declare_dram_parameter("qgam", [P, 2], F32, isOutput=False)
    kgam = nc.declare_dram_parameter("kgam", [P, 2], F32, isOutput=False)
    ones = nc.declare_dram_parameter("ones", [P, 1], F32R, isOutput=False)
    ident = nc.declare_dram_parameter("ident", [P, P], F32, isOutput=False)
    y = nc.declare_dram_parameter("y", [L, IDIM], F32, isOutput=True)

    with TileContext(nc) as tc:
        with tc.tile_pool(name="persist", bufs=1) as persist, \
             tc.tile_pool(name="xts", bufs=3) as xts, \
             tc.tile_pool(name="scr", bufs=3) as scr, \
             tc.tile_pool(name="stats", bufs=4) as stats, \
             tc.tile_pool(name="rope", bufs=2) as ropep:

            # persistent tensors
            kT_all = persist.tile([P, 2, L], F32R)     # 32KB/part
            v_all = persist.tile([P, nL, D], F32R)     # 32KB/part
            wqg_sb = persist.tile([P, nKO, 2 * D], F32R)
            wo_sb = persist.tile([P, 2, IDIM], F32R)
            ident_sb = persist.tile([P, P], F32)
            ones_sb = persist.tile([P, 1], F32R)
            qgam_sb = persist.tile([P, 2], F32)
            kgam_sb = persist.tile([P, 2], F32)
            eps_k = persist.tile([P, 1], F32)
            eps_q = persist.tile([P, 1], F32)
            nc.gpsimd.memset(eps_k[:], EPS)
            nc.gpsimd.memset(eps_q[:], D * EPS)
            nc.sync.dma_start(wqg_sb[:], wqg.rearrange("(ko p) n -> p ko n", p=P))
            nc.sync.dma_start(wo_sb[:], wo.rearrange("(dc p) n -> p dc n", p=P))
            nc.sync.dma_start(ident_sb[:], ident[:, :])
            nc.sync.dma_start(ones_sb[:], ones[:, :])
            nc.sync.dma_start(qgam_sb[:], qgam[:, :])
            nc.sync.dma_start(kgam_sb[:], kgam[:, :])

            def norm_rope(psum_in, cos_t, sin_t, sqrt_scale, sqrt_bias):
                """rmsnorm + partial rope on a [P, D] natural-layout psum tile.
                Returns normed+roped [P, D] f32 sbuf tile."""
                sq = scr.tile([P, D], F32, tag="sq")
                nc.scalar.activation(sq[:], psum_in, AF.Square)
                ssq = stats.tile([P, 1], F32, tag="ssq")
                nc.vector.reduce_sum(ssq[:], sq[:], axis=AX.X)
                std = stats.tile([P, 1], F32, tag="std")
                nc.scalar.activation(std[:], ssq[:], AF.Sqrt,
                                     scale=sqrt_scale, bias=sqrt_bias)
                rstd = stats.tile([P, 1], F32, tag="rstd")
                nc.vector.reciprocal(rstd[:], std[:])
                xn = scr.tile([P, D], F32, tag="xn")
                nc.vector.tensor_scalar_mul(xn[:], psum_in, rstd[:])
                # rope on first 64 cols: xn' = xn*cos + rot(xn)*sin
                t1 = ropep.tile([P, ROPE], F32, tag="t1")
                nc.vector.tensor_mul(t1[:], xn[:, 0:ROPE], cos_t)
                rot = ropep.tile([P, ROPE], F32, tag="rot")
                nc.vector.tensor_scalar_mul(rot[:, 0:32], xn[:, 32:64], -1.0)
                nc.vector.tensor_copy(rot[:, 32:64], xn[:, 0:32])
                t2 = ropep.tile([P, ROPE], F32, tag="t2")
                nc.vector.tensor_mul(t2[:], rot[:], sin_t)
                nc.vector.tensor_add(xn[:, 0:ROPE], t1[:], t2[:])
                return xn

            # ---------------- phase A: k/v for all L ----------------
            with tc.tile_pool(name="wkv", bufs=1) as wkvp, \
                 tc.tile_pool(name="pk", bufs=2, space="PSUM") as pkp, \
                 tc.tile_pool(name="pv", bufs=2, space="PSUM") as pvp, \
                 tc.tile_pool(name="ptrA", bufs=2, space="PSUM") as ptrA:
                wk_sb = wkvp.tile([P, nKO, D], F32R)
                wv_sb = wkvp.tile([P, nKO, D], F32R)
                nc.sync.dma_start(wk_sb[:], wk.rearrange("(ko p) n -> p ko n", p=P))
                nc.sync.dma_start(wv_sb[:], wv.rearrange("(ko p) n -> p ko n", p=P))

                for t in range(nL):
                    xt = xts.tile([P, nKO, P], F32R, tag="xt")
                    nc.sync.dma_start(
                        xt[:], xT[:, t * P:(t + 1) * P].rearrange(
                            "(ko p) l -> p ko l", p=P))
                    psum_k = pkp.tile([P, D], F32)
                    psum_v = pvp.tile([P, D], F32)
                    for ko in range(nKO):
                        nc.tensor.matmul(psum_k[:], xt[:, ko], wk_sb[:, ko],
                                         start=(ko == 0), stop=(ko == nKO - 1))
                    for ko in range(nKO):
                        nc.tensor.matmul(psum_v[:], xt[:, ko], wv_sb[:, ko],
                                         start=(ko == 0), stop=(ko == nKO - 1))
                    nc.vector.tensor_copy(v_all[:, t], psum_v[:])
                    ck = ropep.tile([P, ROPE], F32, tag="ck")
                    sk = ropep.tile([P, ROPE], F32, tag="sk")
                    nc.sync.dma_start(ck[:], cosk[t * P:(t + 1) * P, :])
                    nc.sync.dma_start(sk[:], sink[t * P:(t + 1) * P, :])
                    kn = norm_rope(psum_k[:], ck[:], sk[:], 1.0 / D, eps_k[:])
                    for dc in range(2):
                        ptr = ptrA.tile([P, P], F32, tag="ptr")
                        nc.tensor.transpose(ptr[:], kn[:, dc * P:(dc + 1) * P],
                                            ident_sb[:])
                        nc.vector.tensor_scalar_mul(
                            kT_all[:, dc, t * P:(t + 1) * P], ptr[:],
                            kgam_sb[:, dc:dc + 1])

            # ---------------- phase B: per query tile ----------------
            with tc.tile_pool(name="qgp", bufs=2, space="PSUM") as qgyp, \
                 tc.tile_pool(name="ptrB", bufs=1, space="PSUM") as ptrB, \
                 tc.tile_pool(name="ps", bufs=2, space="PSUM") as psp, \
                 tc.tile_pool(name="pot", bufs=1, space="PSUM") as potp, \
                 tc.tile_pool(name="pden", bufs=1, space="PSUM") as pdenp, \
                 tc.tile_pool(name="qt", bufs=2) as qtp, \
                 tc.tile_pool(name="es", bufs=3) as esp, \
                 tc.tile_pool(name="og", bufs=2) as ogp, \
                 tc.tile_pool(name="yo", bufs=3) as yop:
                for qt in range(nQT):
                    qT_t = qtp.tile([P, 2, TQ], F32R, tag="qT")
                    sgT_t = qtp.tile([P, 2, TQ], F32, tag="sgT")
                    for ls in range(nLS):
                        t = qt * nLS + ls
                        xt = xts.tile([P, nKO, P], F32R, tag="xt")
                        nc.sync.dma_start(
                            xt[:], xT[:, t * P:(t + 1) * P].rearrange(
                                "(ko p) l -> p ko l", p=P))
                        psum_qg = qgyp.tile([P, 2 * D], F32, tag="qg")
                        for ko in range(nKO):
                            nc.tensor.matmul(psum_qg[:], xt[:, ko], wqg_sb[:, ko],
                                             start=(ko == 0), stop=(ko == nKO - 1))
                        # gate -> sigmoid -> transpose
                        sg = scr.tile([P, D], F32, tag="sg")
                        nc.scalar.activation(sg[:], psum_qg[:, D:2 * D], AF.Sigmoid)
                        for dc in range(2):
                            ptr = ptrB.tile([P, P], F32, tag="ptr")
                            nc.tensor.transpose(ptr[:], sg[:, dc * P:(dc + 1) * P],
                                                ident_sb[:])
                            nc.vector.tensor_copy(
                                sgT_t[:, dc, ls * P:(ls + 1) * P], ptr[:])
                        # q: rmsnorm (with /16 folded) + rope + transpose
                        cq = ropep.tile([P, ROPE], F32, tag="cq")
                        sq_ = ropep.tile([P, ROPE], F32, tag="sq_")
                        nc.sync.dma_start(cq[:], cosq[t * P:(t + 1) * P, :])
                        nc.sync.dma_start(sq_[:], sinq[t * P:(t + 1) * P, :])
                        # sqrt(ssq + D*eps) = sqrt(D) * sqrt(mean+eps); recip
                        # gives rstd/16 since sqrt(D)=16
                        qn = norm_rope(psum_qg[:, 0:D], cq[:], sq_[:], 1.0, eps_q[:])
                        for dc in range(2):
                            ptr = ptrB.tile([P, P], F32, tag="ptr")
                            nc.tensor.transpose(ptr[:], qn[:, dc * P:(dc + 1) * P],
                                                ident_sb[:])
                            nc.vector.tensor_scalar_mul(
                                qT_t[:, dc, ls * P:(ls + 1) * P], ptr[:],
                                qgam_sb[:, dc:dc + 1])

                    # attention over kv chunks
                    n_kc = (qt + 1) * nLS if causal else nL
                    psum_ot = [potp.tile([P, TQ], F32, tag=f"ot{dc}", name=f"ot{dc}")
                               for dc in range(2)]
                    psum_den = pdenp.tile([1, TQ], F32, tag="den")
                    for kc in range(n_kc):
                        psum_s = psp.tile([P, TQ], F32, tag="s")
                        for dc in range(2):
                            nc.tensor.matmul(
                                psum_s[:], kT_all[:, dc, kc * P:(kc + 1) * P],
                                qT_t[:, dc], start=(dc == 0), stop=(dc == 1))
                        es = esp.tile([P, TQ], F32R, tag="es")
                        nc.scalar.activation(es[:], psum_s[:], AF.Exp)
                        if causal and kc >= qt * nLS:
                            j = kc - qt * nLS
                            # keep where qf - p - 128*j >= 0 else 0
                            nc.gpsimd.affine_select(
                                out=es[:], in_=es[:], compare_op=OP.is_ge,
                                fill=0.0, base=-P * j,
                                pattern=[[1, TQ]], channel_multiplier=-1)
                        nc.tensor.matmul(psum_den[:], ones_sb[:],
                                         es[:].rearrange("k (ls p) -> k p ls", p=P),
                                         start=(kc == 0), stop=(kc == n_kc - 1))
                        for dc in range(2):
                            nc.tensor.matmul(
                                psum_ot[dc][:], v_all[:, kc, dc * P:(dc + 1) * P],
                                es[:], start=(kc == 0), stop=(kc == n_kc - 1))

                    # denominators -> [P, nLS] reciprocal via dram bounce
                    den_sb = stats.tile([1, TQ], F32, tag="densb")
                    nc.vector.tensor_copy(den_sb[:], psum_den[:])
                    den_in = stats.tile([P, nLS], F32, tag="denin")
                    nc.sync.dma_start(den_in[:, :], den_sb[0:1, :])
                    recip = stats.tile([P, nLS], F32, tag="recip")
                    nc.vector.reciprocal(recip[:], den_in[:])

                    # gate the attention output
                    og = [ogp.tile([P, TQ], F32R, tag=f"og{dc}", name=f"og{dc}")
                          for dc in range(2)]
                    for dc in range(2):
                        nc.vector.tensor_mul(og[dc][:], psum_ot[dc][:],
                                             sgT_t[:, dc])

                    # out-projection, 1/den folded into copy scale
                    for ls in range(nLS):
                        for oc in range(IDIM // TQ):
                            psum_y = psp.tile([P, TQ], F32, tag="s", name="psum_y")
                            for dc in range(2):
                                nc.tensor.matmul(
                                    psum_y[:], og[dc][:, ls * P:(ls + 1) * P],
                                    wo_sb[:, dc, oc * TQ:(oc + 1) * TQ],
                                    start=(dc == 0), stop=(dc == 1))
                            y_sb = yop.tile([P, TQ], F32, tag="ysb")
                            nc.vector.tensor_scalar_mul(y_sb[:], psum_y[:],
                                                        recip[:, ls:ls + 1])
                            nc.sync.dma_start(
                                y[qt * TQ + ls * P: qt * TQ + (ls + 1) * P,
                                  oc * TQ:(oc + 1) * TQ], y_sb[:])
    nc.compile()
    return nc


def _get_nc(L, causal):
    key = (L, causal)
    if key not in _nc_cache:
        _nc_cache[key] = build_nc(L, causal)
    return _nc_cache[key]


def kernel(x, cos, sin, mask, wq, wk, wv, wo, q_gamma, k_gamma):
    B, L, _ = x.shape
    n_heads = 8
    group_size = 4

    mask = np.asarray(mask)
    causal_ref = np.triu(np.ones((L, L), dtype=bool), k=1)
    if np.array_equal(mask, causal_ref):
        causal = True
    elif not mask.any():
        causal = False
    else:
        raise NotImplementedError("only causal or empty masks supported")

    nc = _get_nc(L, causal)

    xT = np.ascontiguousarray(np.asarray(x[0]).T, dtype=np.float32)
    cos = np.asarray(cos, dtype=np.float32)
    sin = np.asarray(sin, dtype=np.float32)
    qg = np.asarray(q_gamma, dtype=np.float32)
    kg = np.asarray(k_gamma, dtype=np.float32)
    idx = (np.arange(ROPE) + 32) % ROPE

    def fold(g):
        cosA = np.ascontiguousarray(cos * g[None, :ROPE])
        sinA = np.ascontiguousarray(sin * g[idx][None, :])
        gam = np.empty((P, 2), dtype=np.float32)
        gam[:, 0] = np.concatenate([np.ones(ROPE, np.float32), g[ROPE:P]])
        gam[:, 1] = g[P:D]
        return cosA, sinA, gam

    cosq_h, sinq_h, qgam_h = fold(qg)
    cosk_h, sink_h, kgam_h = fold(kg)
    ones_h = np.ones((P, 1), dtype=np.float32)
    ident_h = np.eye(P, dtype=np.float32)

    in_maps = []
    for c in range(n_heads):
        g = c // group_size
        in_maps.append({
            "xT": xT,
            "wqg": np.ascontiguousarray(wq[:, c * 2 * D:(c + 1) * 2 * D]),
            "wk": np.ascontiguousarray(wk[:, g * D:(g + 1) * D]),
            "wv": np.ascontiguousarray(wv[:, g * D:(g + 1) * D]),
            "wo": np.ascontiguousarray(wo[c * D:(c + 1) * D, :]),
            "cosq": cosq_h, "sinq": sinq_h,
            "cosk": cosk_h, "sink": sink_h,
            "qgam": qgam_h, "kgam": kgam_h,
            "ones": ones_h, "ident": ident_h,
        })
    global _last_in_maps
    _last_in_maps = in_maps
    res = run_bass_kernel_spmd(nc, in_maps, core_ids=list(range(n_heads)))
    out = np.zeros((L, IDIM), dtype=np.float64)
    for c in range(n_heads):
        out += res.results[c]["y"]
    return out.astype(np.float32).reshape(B, L, IDIM)


# revision 16
# speedup vs baseline: 1.1482x; 1.0983x over previous
"""GQA attention kernel for Trainium2, 8 NeuronCores, head-per-core sharding.

Per core c (head h=c, kv-group g=c//4):
  phase A: k/v projections for all L rows (k: rmsnorm+rope, transposed to kT)
  phase B: per 512-query tile: q/gate projection (rmsnorm+rope+sigmoid,
           transposed), then S^T = kT.T@qT flash attention with exp on ACT,
           causal fill via gpsimd affine_select, denominator via ones-matmul,
           AV into OT, gating, out-projection with 1/denom folded into the
           PSUM->SBUF copy scale. Host sums the 8 partial outputs.
All matmuls run in float32r (1 cycle/row at moving-dim>=256).
"""
import sys
sys.path.insert(0, '/opt/trn_rl_repo')
import numpy as np
import concourse.bacc as bacc
import concourse.mybir as mybir
from concourse.tile import TileContext
from concourse.bass_utils import run_bass_kernel_spmd

P = 128
IDIM = 2048
D = 256           # head dim
ROPE = 64
TQ = 512          # queries per attention tile
EPS = 1e-6
F32 = mybir.dt.float32
F32R = mybir.dt.float32r
AF = mybir.ActivationFunctionType
AX = mybir.AxisListType
OP = mybir.AluOpType

_nc_cache = {}
_last_in_maps = None


def build_nc(L, causal):
    nc = bacc.Bacc()
    nKO = IDIM // P   # 16 idim chunks
    nL = L // P       # 128-row L tiles
    nQT = L // TQ     # query tiles
    nLS = TQ // P     # 4 L-subtiles per query tile

    xT = nc.declare_dram_parameter("xT", [IDIM, L], F32R, isOutput=False)
    wqg = nc.declare_dram_parameter("wqg", [IDIM, 2 * D], F32R, isOutput=False)
    wkv = nc.declare_dram_parameter("wkv", [IDIM, 2 * D], F32R, isOutput=False)
    wo = nc.declare_dram_parameter("wo", [D, IDIM], F32R, isOutput=False)
    cosq = nc.declare_dram_parameter("cosq", [L, ROPE], F32, isOutput=False)
    sinq = nc.declare_dram_parameter("sinq", [L, ROPE], F32, isOutput=False)
    cosk = nc.declare_dram_parameter("cosk", [L, ROPE], F32, isOutput=False)
    sink = nc.declare_dram_parameter("sink", [L, ROPE], F32, isOutput=False)
    qgam = nc.declare_dram_parameter("qgam", [P, 2], F32, isOutput=False)
    kgam = nc.declare_dram_parameter("kgam", [P, 2], F32, isOutput=False)
    ones = nc.declare_dram_parameter("ones", [P, 1], F32R, isOutput=False)
    ident = nc.declare_dram_parameter("ident", [P, P], F32, isOutput=False)
    y = nc.declare_dram_parameter("y", [L, IDIM], F32, isOutput=True)

    with TileContext(nc) as tc:
        with tc.tile_pool(name="persist", bufs=1) as persist, \
             tc.tile_pool(name="xts", bufs=3) as xts, \
             tc.tile_pool(name="scr", bufs=4) as scr, \
             tc.tile_pool(name="stats", bufs=4) as stats, \
             tc.tile_pool(name="rope", bufs=2) as ropep:

            # persistent tensors
            kT_all = persist.tile([P, 2, L], F32R)     # 32KB/part
            v_all = persist.tile([P, nL, D], F32R)     # 32KB/part
            wqg_sb = persist.tile([P, nKO, 2 * D], F32R)
            wo_sb = persist.tile([P, 2, IDIM], F32R)
            ident_sb = persist.tile([P, P], F32)
            ones_sb = persist.tile([P, 1], F32R)
            qgam_sb = persist.tile([P, 2], F32)
            kgam_sb = persist.tile([P, 2], F32)
            eps_k = persist.tile([P, 1], F32)
            eps_q = persist.tile([P, 1], F32)
            nc.gpsimd.memset(eps_k[:], EPS)
            nc.gpsimd.memset(eps_q[:], D * EPS)
            nc.sync.dma_start(ident_sb[:], ident[:, :])
            nc.sync.dma_start(kgam_sb[:], kgam[:, :])
            nc.sync.dma_start(qgam_sb[:], qgam[:, :])

            def norm_rope(psum_in, cos_t, sin_t, sqrt_scale, sqrt_bias):
                """rmsnorm + partial rope on a [P, D] natural-layout psum tile.
                Returns normed+roped [P, D] f32 sbuf tile."""
                sq = scr.tile([P, D], F32, tag="sq")
                nc.scalar.activation(sq[:], psum_in, AF.Square)
                ssq = stats.tile([P, 1], F32, tag="ssq")
                nc.vector.reduce_sum(ssq[:], sq[:], axis=AX.X)
                std = stats.tile([P, 1], F32, tag="std")
                nc.scalar.activation(std[:], ssq[:], AF.Sqrt,
                                     scale=sqrt_scale, bias=sqrt_bias)
                rstd = stats.tile([P, 1], F32, tag="rstd")
                nc.vector.reciprocal(rstd[:], std[:])
                xn = scr.tile([P, D], F32, tag="xn")
                nc.vector.tensor_scalar_mul(xn[:], psum_in, rstd[:])
                # rope on first 64 cols: xn' = xn*cos + rot(xn)*sin
                t1 = ropep.tile([P, ROPE], F32, tag="t1")
                nc.vector.tensor_mul(t1[:], xn[:, 0:ROPE], cos_t)
                rot = ropep.tile([P, ROPE], F32, tag="rot")
                nc.vector.tensor_scalar_mul(rot[:, 0:32], xn[:, 32:64], -1.0)
                nc.vector.tensor_copy(rot[:, 32:64], xn[:, 0:32])
                t2 = ropep.tile([P, ROPE], F32, tag="t2")
                nc.vector.tensor_mul(t2[:], rot[:], sin_t)
                nc.vector.tensor_add(xn[:, 0:ROPE], t1[:], t2[:])
                return xn

            # ---------------- phase A: k/v for all L ----------------
            with tc.tile_pool(name="wkv", bufs=1) as wkvp, \
                 tc.tile_pool(name="pk", bufs=3, space="PSUM") as pkp, \
                 tc.tile_pool(name="ptrA", bufs=2, space="PSUM") as ptrA:
                wkv_sb = wkvp.tile([P, nKO, 2 * D], F32R)
                nc.sync.dma_start(wkv_sb[:], wkv.rearrange("(ko p) n -> p ko n", p=P))

                for t in range(nL):
                    if t == 1:
                        nc.sync.dma_start(
                            wqg_sb[:], wqg.rearrange("(ko p) n -> p ko n", p=P))
                    if t == 2:
                        nc.sync.dma_start(
                            wo_sb[:], wo.rearrange("(dc p) n -> p dc n", p=P))
                        nc.sync.dma_start(ones_sb[:], ones[:, :])
                    xt = xts.tile([P, nKO, P], F32R, tag="xt")
                    nc.sync.dma_start(
                        xt[:], xT[:, t * P:(t + 1) * P].rearrange(
                            "(ko p) l -> p ko l", p=P))
                    psum_kv = pkp.tile([P, 2 * D], F32)
                    for ko in range(nKO):
                        nc.tensor.matmul(psum_kv[:], xt[:, ko], wkv_sb[:, ko],
                                         start=(ko == 0), stop=(ko == nKO - 1))
                    nc.vector.tensor_copy(v_all[:, t], psum_kv[:, D:2 * D])
                    ck = ropep.tile([P, ROPE], F32, tag="ck")
                    sk = ropep.tile([P, ROPE], F32, tag="sk")
                    nc.sync.dma_start(ck[:], cosk[t * P:(t + 1) * P, :])
                    nc.sync.dma_start(sk[:], sink[t * P:(t + 1) * P, :])
                    kn = norm_rope(psum_kv[:, 0:D], ck[:], sk[:], 1.0 / D, eps_k[:])
                    for dc in range(2):
                        ptr = ptrA.tile([P, P], F32, tag="ptr")
                        nc.tensor.transpose(ptr[:], kn[:, dc * P:(dc + 1) * P],
                                            ident_sb[:])
                        nc.vector.tensor_scalar_mul(
                            kT_all[:, dc, t * P:(t + 1) * P], ptr[:],
                            kgam_sb[:, dc:dc + 1])

            # ---------------- phase B: per query tile ----------------
            with tc.tile_pool(name="qgp", bufs=2, space="PSUM") as qgyp, \
                 tc.tile_pool(name="ptrB", bufs=1, space="PSUM") as ptrB, \
                 tc.tile_pool(name="ps", bufs=2, space="PSUM") as psp, \
                 tc.tile_pool(name="pot", bufs=1, space="PSUM") as potp, \
                 tc.tile_pool(name="pden", bufs=1, space="PSUM") as pdenp, \
                 tc.tile_pool(name="qt", bufs=2) as qtp, \
                 tc.tile_pool(name="es", bufs=4) as esp, \
                 tc.tile_pool(name="og", bufs=2) as ogp, \
                 tc.tile_pool(name="dac", bufs=2) as dacp, \
                 tc.tile_pool(name="yo", bufs=3) as yop:
                for qt in range(nQT):
                    qT_t = qtp.tile([P, 2, TQ], F32R, tag="qT")
                    sgT_t = qtp.tile([P, 2, TQ], F32, tag="sgT")
                    for ls in range(nLS):
                        t = qt * nLS + ls
                        xt = xts.tile([P, nKO, P], F32R, tag="xt")
                        nc.sync.dma_start(
                            xt[:], xT[:, t * P:(t + 1) * P].rearrange(
                                "(ko p) l -> p ko l", p=P))
                        psum_qg = qgyp.tile([P, 2 * D], F32, tag="qg")
                        for ko in range(nKO):
                            nc.tensor.matmul(psum_qg[:], xt[:, ko], wqg_sb[:, ko],
                                             start=(ko == 0), stop=(ko == nKO - 1))
                        # gate -> sigmoid -> transpose
                        sg = scr.tile([P, D], F32, tag="sg")
                        nc.scalar.activation(sg[:], psum_qg[:, D:2 * D], AF.Sigmoid)
                        for dc in range(2):
                            ptr = ptrB.tile([P, P], F32, tag="ptr")
                            nc.tensor.transpose(ptr[:], sg[:, dc * P:(dc + 1) * P],
                                                ident_sb[:])
                            nc.vector.tensor_copy(
                                sgT_t[:, dc, ls * P:(ls + 1) * P], ptr[:])
                        # q: rmsnorm (with /16 folded) + rope + transpose
                        cq = ropep.tile([P, ROPE], F32, tag="cq")
                        sq_ = ropep.tile([P, ROPE], F32, tag="sq_")
                        nc.sync.dma_start(cq[:], cosq[t * P:(t + 1) * P, :])
                        nc.sync.dma_start(sq_[:], sinq[t * P:(t + 1) * P, :])
                        # sqrt(ssq + D*eps) = sqrt(D) * sqrt(mean+eps); recip
                        # gives rstd/16 since sqrt(D)=16
                        qn = norm_rope(psum_qg[:, 0:D], cq[:], sq_[:], 1.0, eps_q[:])
                        for dc in range(2):
                            ptr = ptrB.tile([P, P], F32, tag="ptr")
                            nc.tensor.transpose(ptr[:], qn[:, dc * P:(dc + 1) * P],
                                                ident_sb[:])
                            nc.vector.tensor_scalar_mul(
                                qT_t[:, dc, ls * P:(ls + 1) * P], ptr[:],
                                qgam_sb[:, dc:dc + 1])

                    # attention over kv chunks
                    n_kc = (qt + 1) * nLS if causal else nL
                    psum_ot = [potp.tile([P, TQ], F32, tag=f"ot{dc}", name=f"ot{dc}")
                               for dc in range(2)]
                    den_acc = dacp.tile([P, TQ], F32R, tag="denacc")
                    for kc in range(n_kc):
                        psum_s = psp.tile([P, TQ], F32, tag="s")
                        for dc in range(2):
                            nc.tensor.matmul(
                                psum_s[:], kT_all[:, dc, kc * P:(kc + 1) * P],
                                qT_t[:, dc], start=(dc == 0), stop=(dc == 1))
                        es = esp.tile([P, TQ], F32R, tag="es")
                        nc.scalar.activation(es[:], psum_s[:], AF.Exp)
                        if causal and kc >= qt * nLS:
                            j = kc - qt * nLS
                            # keep where qf - p - 128*j >= 0 else 0
                            nc.gpsimd.affine_select(
                                out=es[:], in_=es[:], compare_op=OP.is_ge,
                                fill=0.0, base=-P * j,
                                pattern=[[1, TQ]], channel_multiplier=-1)
                        if kc == 0:
                            nc.vector.tensor_copy(den_acc[:], es[:].bitcast(F32))
                        else:
                            nc.vector.tensor_add(den_acc[:],
                                                 den_acc[:].bitcast(F32),
                                                 es[:].bitcast(F32))
                        for dc in range(2):
                            nc.tensor.matmul(
                                psum_ot[dc][:], v_all[:, kc, dc * P:(dc + 1) * P],
                                es[:], start=(kc == 0), stop=(kc == n_kc - 1))

                    # denominators -> [P, nLS] reciprocal via dram bounce
                    psum_den = pdenp.tile([1, TQ], F32, tag="den")
                    nc.tensor.matmul(psum_den[:], ones_sb[:],
                                     den_acc[:].rearrange("k (ls p) -> k p ls", p=P),
                                     start=True, stop=True)
                    den_sb = stats.tile([1, TQ], F32, tag="densb")
                    nc.vector.tensor_copy(den_sb[:], psum_den[:])
                    den_in = stats.tile([P, nLS], F32, tag="denin")
                    nc.sync.dma_start(den_in[:, :], den_sb[0:1, :])
                    recip = stats.tile([P, nLS], F32, tag="recip")
                    nc.vector.reciprocal(recip[:], den_in[:])

                    # gate the attention output
                    og = [ogp.tile([P, TQ], F32R, tag=f"og{dc}", name=f"og{dc}")
                          for dc in range(2)]
                    for dc in range(2):
                        nc.vector.tensor_mul(og[dc][:], psum_ot[dc][:],
                                             sgT_t[:, dc])

                    # out-projection, 1/den folded into copy scale
                    for ls in range(nLS):
                        for oc in range(IDIM // TQ):
                            psum_y = psp.tile([P, TQ], F32, tag="s", name="psum_y")
                            for dc in range(2):
                                nc.tensor.matmul(
                                    psum_y[:], og[dc][:, ls * P:(ls + 1) * P],
                                    wo_sb[:, dc, oc * TQ:(oc + 1) * TQ],
                                    start=(dc == 0), stop=(dc == 1))
                            y_sb = yop.tile([P, TQ], F32, tag="ysb")
                            nc.vector.tensor_scalar_mul(y_sb[:], psum_y[:],
                                                        recip[:, ls:ls + 1])
                            nc.sync.dma_start(
                                y[qt * TQ + ls * P: qt * TQ + (ls + 1) * P,
                                  oc * TQ:(oc + 1) * TQ], y_sb[:])
    nc.compile()
    return nc


def _get_nc(L, causal):
    key = (L, causal)
    if key not in _nc_cache:
        _nc_cache[key] = build_nc(L, causal)
    return _nc_cache[key]


def kernel(x, cos, sin, mask, wq, wk, wv, wo, q_gamma, k_gamma):
    B, L, _ = x.shape
    n_heads = 8
    group_size = 4

    mask = np.asarray(mask)
    causal_ref = np.triu(np.ones((L, L), dtype=bool), k=1)
    if np.array_equal(mask, causal_ref):
        causal = True
    elif not mask.any():
        causal = False
    else:
        raise NotImplementedError("only causal or empty masks supported")

    nc = _get_nc(L, causal)

    xT = np.ascontiguousarray(np.asarray(x[0]).T, dtype=np.float32)
    cos = np.asarray(cos, dtype=np.float32)
    sin = np.asarray(sin, dtype=np.float32)
    qg = np.asarray(q_gamma, dtype=np.float32)
    kg = np.asarray(k_gamma, dtype=np.float32)
    idx = (np.arange(ROPE) + 32) % ROPE

    def fold(g):
        cosA = np.ascontiguousarray(cos * g[None, :ROPE])
        sinA = np.ascontiguousarray(sin * g[idx][None, :])
        gam = np.empty((P, 2), dtype=np.float32)
        gam[:, 0] = np.concatenate([np.ones(ROPE, np.float32), g[ROPE:P]])
        gam[:, 1] = g[P:D]
        return cosA, sinA, gam

    cosq_h, sinq_h, qgam_h = fold(qg)
    cosk_h, sink_h, kgam_h = fold(kg)
    ones_h = np.ones((P, 1), dtype=np.float32)
    ident_h = np.eye(P, dtype=np.float32)

    in_maps = []
    for c in range(n_heads):
        g = c // group_size
        in_maps.append({
            "xT": xT,
            "wqg": np.ascontiguousarray(wq[:, c * 2 * D:(c + 1) * 2 * D]),
            "wkv": np.ascontiguousarray(np.concatenate(
                [wk[:, g * D:(g + 1) * D], wv[:, g * D:(g + 1) * D]], axis=1)),
            "wo": np.ascontiguousarray(wo[c * D:(c + 1) * D, :]),
            "cosq": cosq_h, "sinq": sinq_h,
            "cosk": cosk_h, "sink": sink_h,
            "qgam": qgam_h, "kgam": kgam_h,
            "ones": ones_h, "ident": ident_h,
        })
    global _last_in_maps
    _last_in_maps = in_maps
    res = run_bass_kernel_spmd(nc, in_maps, core_ids=list(range(n_heads)))
    out = np.zeros((L, IDIM), dtype=np.float64)
    for c in range(n_heads):
        out += res.results[c]["y"]
    return out.astype(np.float32).reshape(B, L, IDIM)
